# revision 3
# baseline (speedup 1.0000x reference)
"""Trainium2 Bass kernel for nn_BonzSelfAttention — v5.

Data-parallel over batch: B=8 -> 8 NeuronCores. Per-core (x: [N=2048, D=768]):

Design (cold-PE-aware: minimize PE cycles, keep phases dense):
- proj (x^T@pk, bf16) -> xpT; kconv (bf16) -> kproj; vaug = [64 ones | 32*V]
  fp8 built on DVE.
- qconv via fp8 DoubleRow: host packs xt/wq grouped-paired [96, g, 2, .]
  so contract 192 runs in one DR pass (fp8 touches Q only).
- Attention: 24 slots = 6 head-pairs x 4 n-blocks(512), block-major.
  Pair-tile dt [128,4,512] (4 psum banks, pool bufs=2):
    dots: 4 bf16 MMs, row-tiled across the pair (bases 0/64).
    exp:  ONE ACT instr [128,4,512] -> PT fp8.
    ctx:  2 DR MMs (contract 256 = k2-pairs) into banks 0-1:
          rows 0-63 = softmax sums (64 ones-cols), 64-127 = 32*ctx.
    norm: one recip [64,2,512] (base-0: custom-DVE base-partition erratum!)
          + 2 DVE muls -> ctxT fp8.
- Out-proj chunk: 6 DR MMs (32*ctx @ 32*w_out.T) + identity-MM adding
  1024*(x+b_out) (host tensor xab) => psum op = 1024*y. LN is
  scale-invariant so 1024 cancels. Evacuate+row-sum via ACT Identity
  with accum_out (no DVE pass). A few chunks run as B fillers in banks
  2-3 of the retiring pair-tile; the rest form a dense PE run in C.
- LN: ACT Square(bias=-mu) accum -> ssq; rstd = Exp(-.5*Ln(ssq/D+eps))
  (Ln+Exp+Square+Identity live in one table set -> zero table switches);
  of = (y'+negmu')*rstd' on DVE/gpsimd; bf16 out, host upcast.
"""
import sys

if "/opt/trn_rl_repo" not in sys.path:
    sys.path.insert(0, "/opt/trn_rl_repo")

import os
from contextlib import ExitStack

import ml_dtypes
import numpy as np

import concourse.bass as bass
import concourse.bacc as bacc
import concourse.mybir as mybir
import concourse.tile as tile
from concourse.bass_utils import run_bass_kernel_spmd

FP = mybir.dt.float32
BF = mybir.dt.bfloat16
F8 = mybir.dt.float8e4
AF = mybir.ActivationFunctionType
ALU = mybir.AluOpType
PM = mybir.MatmulPerfMode

B, N, D = 8, 2048, 768
K, H, G = 256, 12, 4
DH, GD = 64, 192
EPS = 1e-12
NCORES = 8

CTX_SCALE = 32.0
WOS_SCALE = 32.0
YSCALE = CTX_SCALE * WOS_SCALE          # residual pre-scaled by this
N_FILL = int(os.environ.get("BONZ_NFILL", "6"))   # out-proj chunks in B


def g_chunks(g):
    """contraction chunks of group g on the global 128-grid of d."""
    d0 = g * GD
    c = []
    while d0 < (g + 1) * GD:
        t, p = d0 // 128, d0 % 128
        sz = min(128 - p, (g + 1) * GD - d0)
        c.append((t, p, sz, d0 - g * GD))
        d0 += sz
    return c


def build_program():
    nc = bacc.Bacc(None, target_bir_lowering=False)
    xa_d = nc.declare_dram_parameter("xa", [128, 16 * D], BF, isOutput=False)
    xb_d = nc.declare_dram_parameter("xab", [128, 16 * D], BF, isOutput=False)
    xt_d = nc.declare_dram_parameter("xtg", [96, G * 2 * N], F8, isOutput=False)
    pk_d = nc.declare_dram_parameter("pk", [128, 16 * K], BF, isOutput=False)
    wq_d = nc.declare_dram_parameter("wqg", [96, G * 2 * GD], F8, isOutput=False)
    wk_d = nc.declare_dram_parameter("wkt", [128, 6 * GD], BF, isOutput=False)
    wo_d = nc.declare_dram_parameter("wos", [128, 6 * D], F8, isOutput=False)
    id_d = nc.declare_dram_parameter("ident", [128, 128], BF, isOutput=False)
    out_d = nc.declare_dram_parameter("out", [N, D], BF, isOutput=True)

    with tile.TileContext(nc) as tc, ExitStack() as top:
        persist = top.enter_context(tc.tile_pool(name="persist", bufs=1))
        xaq = [persist.tile([128, 4, D], BF, name=f"xaq{q}") for q in range(4)]
        xbq = [persist.tile([128, 4, D], BF, name=f"xbq{q}") for q in range(4)]
        pkq = [persist.tile([128, 4, K], BF, name=f"pkq{q}") for q in range(4)]
        xtg = persist.tile([96, G, 2, N], F8)   # x.T grouped-paired
        wqg = persist.tile([96, G, 2, GD], F8)
        wkg = persist.tile([128, 6, GD], BF)
        qT = persist.tile([128, 6, N], BF)
        kproj = persist.tile([128, 2, D], BF)
        ctxT = persist.tile([128, 6, N], F8)    # 32*ctx
        vaug = persist.tile([128, 2 * H, 128], F8)
        wos = persist.tile([128, 6, D], F8)     # 32*w_out.T
        ident = persist.tile([128, 128], BF)
        yall = persist.tile([128, 16, D], FP)   # 1024*y rows
        nmu = persist.tile([128, 16], FP)
        ssqa = persist.tile([128, 16], FP)
        lnt = persist.tile([128, 16], FP)
        rstda = persist.tile([128, 16], FP)
        epsc = persist.tile([128, 1], FP)
        scr1 = persist.tile([128, 1], FP)
        sqscr = persist.tile([128, D], FP)   # Square scratch (discarded)

        nc.vector.memset(epsc, EPS)
        nc.scalar.activation(scr1, epsc, AF.Exp)   # preload exp table set

        # ---------------- DMAs (host-packed, contiguous per partition) ----
        xa3 = xa_d.rearrange("p (c d) -> p c d", c=16)
        xb3 = xb_d.rearrange("p (c d) -> p c d", c=16)
        pk3 = pk_d.rearrange("p (c k) -> p c k", c=16)
        for q in range(4):
            nc.sync.dma_start(out=pkq[q], in_=pk3[:, 4 * q:4 * q + 4, :])
            nc.sync.dma_start(out=xaq[q], in_=xa3[:, 4 * q:4 * q + 4, :])
        nc.sync.dma_start(
            out=xtg, in_=xt_d.rearrange("p (g j n) -> p g j n", g=G, j=2))
        nc.sync.dma_start(
            out=wqg, in_=wq_d.rearrange("p (g j o) -> p g j o", g=G, j=2))
        nc.sync.dma_start(out=wkg, in_=wk_d.rearrange("p (t o) -> p t o", t=6))
        nc.sync.dma_start(out=ident, in_=id_d[:, :])
        nc.sync.dma_start(out=wos, in_=wo_d.rearrange("p (t c) -> p t c", t=6))
        for q in range(4):
            nc.sync.dma_start(out=xbq[q], in_=xb3[:, 4 * q:4 * q + 4, :])

        # ---------------- proj + kconv + vaug -----------------------------
        with ExitStack() as pha:
            xpts = pha.enter_context(tc.tile_pool(name="xpts", bufs=1))
            xpT = xpts.tile([128, 6, K], BF)
            with ExitStack() as phxp:
                xpps = phxp.enter_context(
                    tc.tile_pool(name="xpps", bufs=1, space="PSUM"))
                xpp = [xpps.tile([128, K], FP, tag=f"xp{t}", name=f"xpp{t}")
                       for t in range(6)]
                for c in range(16):
                    for t in range(6):
                        nc.tensor.matmul(
                            xpp[t],
                            lhsT=xaq[c // 4][:, c % 4, t * 128:(t + 1) * 128],
                            rhs=pkq[c // 4][:, c % 4, :],
                            start=(c == 0), stop=(c == 15),
                        )
                for t in range(6):
                    nc.vector.tensor_copy(xpT[:, t, :], xpp[t])

            with ExitStack() as phkp:
                kpps = phkp.enter_context(
                    tc.tile_pool(name="kpps", bufs=2, space="PSUM"))
                for kc in range(2):
                    for g in range(G):
                        ps = kpps.tile([128, GD], FP, tag="kp")
                        first = True
                        for (it, ip, isz, ilo) in g_chunks(g):
                            nc.tensor.matmul(
                                ps,
                                lhsT=xpT[ip:ip + isz, it,
                                         kc * 128:(kc + 1) * 128],
                                rhs=wkg[ip:ip + isz, it, :],
                                start=first, stop=not first,
                            )
                            first = False
                        nc.vector.tensor_copy(
                            kproj[:, kc, g * GD:(g + 1) * GD], ps)

        # vaug = [64 ones-cols | 32*V]; sums land on partitions 0-63
        nc.vector.memset(vaug[:, :, 0:DH], 1.0)
        for h in range(H):
            for kc in range(2):
                nc.vector.tensor_scalar_mul(
                    vaug[:, 2 * h + kc, DH:128],
                    kproj[:, kc, h * DH:(h + 1) * DH], CTX_SCALE)

        # ---------------- qconv: fp8 DoubleRow, contract 192 in one pass --
        with ExitStack() as phq:
            qps = phq.enter_context(
                tc.tile_pool(name="qps", bufs=4, space="PSUM"))
            for g in range(G):
                for (ot, op_, osz, olo) in g_chunks(g):
                    for blk in range(4):
                        n0 = blk * 512
                        ps = qps.tile([128, 512], FP, tag="qps")
                        nc.tensor.matmul(
                            ps[:osz, :],
                            lhsT=wqg[:, g, :, olo:olo + osz],
                            rhs=xtg[:, g, :, n0:n0 + 512],
                            start=True, stop=True, perf_mode=PM.DoubleRow,
                        )
                        if (g + blk) % 2 == 0:
                            nc.scalar.copy(qT[op_:op_ + osz, ot, n0:n0 + 512],
                                           ps[:osz, :])
                        else:
                            nc.vector.tensor_copy(
                                qT[op_:op_ + osz, ot, n0:n0 + 512],
                                ps[:osz, :])

        # ---------------- attention slots + out-proj ----------------------
        dps = top.enter_context(tc.tile_pool(name="dps", bufs=1, space="PSUM"))
        cps = top.enter_context(tc.tile_pool(name="cps", bufs=2, space="PSUM"))
        pts = top.enter_context(tc.tile_pool(name="pts", bufs=2))
        rreps = top.enter_context(tc.tile_pool(name="rreps", bufs=2))

        def emit_dots_exp(p, b):
            n0 = b * 512
            dt = dps.tile([128, 4, 512], FP, tag="dt", name=f"dt{p}_{b}")
            for k2c in range(2):
                for j in range(2):           # interleave j -> row-tiling
                    h = 2 * p + j
                    base = h // 4 + 384 * k2c
                    nc.tensor.matmul(
                        dt[:, 2 * j + k2c, :],
                        lhsT=kproj[64 * j:64 * j + 64, (h % 4) // 2,
                                   base:base + 382:3],
                        rhs=qT[64 * j:64 * j + 64, p, n0:n0 + 512],
                        start=True, stop=True,
                    )
            pt = pts.tile([128, 4, 512], F8, tag="pt", name=f"pt{p}_{b}")
            nc.scalar.activation(pt, dt, AF.Exp, scale=0.125)
            return dt, pt

        def emit_ctx_norm(p, b, pt):
            cp = cps.tile([128, 2, 512], FP, tag="cp", name=f"cp{p}_{b}")
            for j in range(2):
                nc.tensor.matmul(
                    cp[:, j, :],
                    lhsT=vaug[:, 2 * (2 * p + j):2 * (2 * p + j) + 2, :],
                    rhs=pt[:, 2 * j:2 * j + 2, :],
                    start=True, stop=True, perf_mode=PM.DoubleRow,
                )
            rrep = rreps.tile([64, 2, 512], FP, tag="rrep")
            nc.vector.reciprocal_approx_fast(rrep, cp[0:64, 0:2, :])
            n0 = b * 512
            for j in range(2):
                nc.vector.tensor_mul(
                    ctxT[64 * j:64 * j + 64, p, n0:n0 + 512],
                    cp[64:128, j, :], rrep[:, j, :])

        def emit_outproj(ncn, op):
            """op = 1024*y as [128, 2, 384] psum."""
            for cb in range(2):
                ops = op[:, cb, 0:384]
                c0 = cb * 384
                for u in range(3):
                    nc.tensor.matmul(
                        ops,
                        lhsT=ctxT[:, 2 * u:2 * u + 2,
                                  ncn * 128:(ncn + 1) * 128],
                        rhs=wos[:, 2 * u:2 * u + 2, c0:c0 + 384],
                        start=(u == 0), stop=False, perf_mode=PM.DoubleRow,
                    )
                nc.tensor.matmul(
                    ops, lhsT=ident,
                    rhs=xbq[ncn // 4][:, ncn % 4, c0:c0 + 384],
                    start=False, stop=True,
                )
            # evacuate + row-sum on ACT (Identity = same table set as Exp)
            ysum = nmu[:, ncn:ncn + 1]
            nc.scalar.activation(
                yall[:, ncn, :].rearrange("p (a b) -> p a b", a=2),
                op[:, :, 0:384], AF.Identity, accum_out=ysum)
            nc.vector.tensor_scalar_mul(ysum, ysum, -1.0 / D)

        # block-major slots; ctx/norm of slot s-1 after dots/exp of slot s
        pend = None

        def slot(p, b):
            nonlocal pend
            dt, pt = emit_dots_exp(p, b)
            if pend is not None:
                pp, pb, ppt = pend
                emit_ctx_norm(pp, pb, ppt)
            pend = (p, b, pt)

        for b in range(4):
            for p in range(6):
                slot(p, b)
        pp, pb, ppt = pend
        emit_ctx_norm(pp, pb, ppt)

        # ---------------- phase C: out-proj chunks + LN tails -------------
        with ExitStack() as phc:
            cs = phc.enter_context(tc.tile_pool(name="cstat", bufs=1))
            ofs = phc.enter_context(tc.tile_pool(name="ofs", bufs=3))
            vva = cs.tile([128, 16], FP)
            nt1 = cs.tile([128, 16], FP)
            nt2 = cs.tile([128, 16], FP)

            def rsqrt_group(g4):
                s = slice(4 * g4, 4 * g4 + 4)
                v = nt1[:, s]
                nc.vector.tensor_scalar_mul(v, vva[:, s], 1.0 / D)
                xi = rstda[:, s].bitcast(mybir.dt.int32)
                nc.vector.tensor_scalar(
                    out=xi, in0=v.bitcast(mybir.dt.int32),
                    scalar1=1, scalar2=-1,
                    op0=ALU.logical_shift_right, op1=ALU.bitwise_xor)
                nc.vector.tensor_scalar(
                    out=xi, in0=xi, scalar1=0x5f3759e0, scalar2=None,
                    op0=ALU.add)
                for _ in range(2):
                    t = nt2[:, s]
                    nc.vector.tensor_mul(t, rstda[:, s], rstda[:, s])
                    nc.vector.tensor_mul(t, t, v)
                    nc.vector.tensor_scalar(
                        out=t, in0=t, scalar1=-0.5, scalar2=1.5,
                        op0=ALU.mult, op1=ALU.add)
                    nc.vector.tensor_mul(rstda[:, s], rstda[:, s], t)

            for ncn in range(16):
                op = cps.tile([128, 2, 512], FP, tag="cp", name=f"op{ncn}")
                emit_outproj(ncn, op)
                nc.vector.scalar_tensor_tensor(
                    out=sqscr, in0=yall[:, ncn, :],
                    scalar=nmu[:, ncn:ncn + 1], in1=yall[:, ncn, :],
                    op0=ALU.add, op1=ALU.mult,
                    accum_out=vva[:, ncn:ncn + 1])
                if ncn % 4 == 3:
                    g4 = ncn // 4
                    rsqrt_group(g4)
                    for i in range(4 * g4, 4 * g4 + 4):
                        of = ofs.tile([128, D], BF, tag="of")
                        eng = nc.gpsimd if i % 2 == 0 else nc.vector
                        eng.tensor_scalar(
                            out=of, in0=yall[:, i, :],
                            scalar1=nmu[:, i:i + 1],
                            scalar2=rstda[:, i:i + 1],
                            op0=ALU.add, op1=ALU.mult)
                        nc.sync.dma_start(
                            out=out_d[i * 128:(i + 1) * 128, :], in_=of)

    return nc


_NC_CACHE = None


def _get_nc():
    global _NC_CACHE
    if _NC_CACHE is None:
        nc = build_program()
        if not nc.is_finalized():
            nc.finalize()
        _NC_CACHE = nc
    return _NC_CACHE


def _bf(a):
    return np.ascontiguousarray(a.astype(ml_dtypes.bfloat16))


def _f8(a):
    return np.ascontiguousarray(a.astype(ml_dtypes.float8_e4m3))


def _pack(a, nt):
    f = a.shape[1]
    return np.ascontiguousarray(
        a.reshape(nt, 128, f).transpose(1, 0, 2).reshape(128, nt * f))


def make_in_maps(inputs):
    x = np.asarray(inputs["input_embedding"], np.float32)
    wq = np.asarray(inputs["wq"], np.float32)
    wk = np.asarray(inputs["wk"], np.float32)
    pk = np.asarray(inputs["project_k"], np.float32)
    w_out = np.asarray(inputs["w_out"], np.float32)
    b_out = np.asarray(inputs["b_out"], np.float32)

    # wq grouped-paired: [96, g, j, o] = wq[g][96j+p, o] (wq as (g, o, i))
    wqt = np.transpose(wq, (0, 2, 1))                   # (g, i, o)
    wqg = _f8(wqt.reshape(G, 2, 96, GD).transpose(2, 0, 1, 3)
              .reshape(96, G * 2 * GD))
    wkt = _bf(_pack(np.transpose(wk, (0, 2, 1)).reshape(D, GD), 6))
    wos = _f8(_pack(w_out.T * WOS_SCALE, 6))
    pk_p = _bf(_pack(pk, 16))
    ident = _bf(np.eye(128, dtype=np.float32))

    in_maps = []
    for c in range(NCORES):
        xc = np.ascontiguousarray(x[c])
        xt = xc.T                                       # (768, 2048)
        xtg = _f8(xt.reshape(G, 2, 96, N).transpose(2, 0, 1, 3)
                  .reshape(96, G * 2 * N))
        in_maps.append({
            "xa": _bf(_pack(xc, 16)),
            "xab": _bf(_pack(YSCALE * (xc + b_out[None, :]), 16)),
            "xtg": xtg, "pk": pk_p, "wqg": wqg, "wkt": wkt, "wos": wos,
            "ident": ident,
        })
    return in_maps


def kernel(**inputs):
    gamma = np.asarray(inputs["gamma"], np.float32)
    beta = np.asarray(inputs["beta"], np.float32)
    nc = _get_nc()
    in_maps = make_in_maps(inputs)
    res = run_bass_kernel_spmd(nc, in_maps, list(range(NCORES)))
    outs = np.stack([np.asarray(res.results[c]["out"]).astype(np.float32)
                     for c in range(NCORES)])
    if not (np.all(gamma == 1.0) and np.all(beta == 0.0)):
        outs = outs * gamma[None, None, :] + beta[None, None, :]
    return outs.astype(np.float32)


if __name__ == "__main__":
    nc = build_program()
    print("program built ok")
